# revision 58
# baseline (speedup 1.0000x reference)
"""Causal self-attention with RoPE on 8 Trainium2 NeuronCores.

Problem: B=4, T=2048, C=1024, 16 heads x 64 dim, fp32 reference.

Sharding: 8 cores = (batch b in 0..3) x (head-group g in 0..1, 8 heads each).
Each core computes qkv for its batch/head-slice (column-parallel qkv),
full attention for its 8 heads, and a row-parallel partial projection.
Host sums the two partial projections per batch (the "all-reduce").

Per-core kernel strategy (software-pipelined):
  - Host pre-transposes x and weights so every matmul contraction dim is
    on SBUF partitions. Matmuls in bf16 (fp32 PSUM accum).
  - qkv produced in [t, f] layout; RoPE applied along the free axis
    (fused with PSUM evacuation on DVE); q/k transposed to [d, t]
    head-pair stacks via DMA xbar transposes on the sync HWDGE queue.
    QT/KT are per-pair tiles so score matmuls only depend on their own
    pair's transposes; the prologue emits block-0 transposes pair-major.
  - Scores are computed TRANSPOSED: ST[tk, tq] = kT.T @ qT per head.
    The two heads of a pair run concurrently in the PE array via row
    tiling (K=64 halves), writing one [128, 1024] 2-bank PSUM tile
    (double-buffered so exp(m) overlaps scores(m+1)). On diagonal-
    straddling tiles the free dim is trimmed to the causally valid
    query range (scores, exp, mask and attn@v all shrink).
  - exp on ScalarE in ONE FD<=1024 instruction per pair-step (scale=1/8
    folded in; no max subtraction: |scores|/8 < ~40 << 88, safe range).
  - Causal masking: one gpsimd affine_select over the merged [p,2,q]
    view zeroes tk > tq for both heads; attn@v lags exp by two m-steps
    so exp+mask latency is off the PE's critical path.
  - attn@v: outT[d, tq] = v.T @ PT with a ones-column appended to v, so
    row 64 accumulates the softmax denominator l for free.
  - Normalization: attn@v PSUM staged to SBUF with one ScalarE copy at
    pair end (frees the banks in ~1us); the l-row copy / reciprocal /
    gpsimd partition broadcast / ONORM multiply are DEFERRED and spread
    over the following pair's m-steps, so every engine reaches them
    with dependencies already satisfied (no cross-FIFO blocking).
  - proj: row-parallel y_partial = ONORM.T @ wprojT, DVE evacuation.
  - Software pipeline: qkv units of block j+1 (front-loaded so their
    transposes enqueue early on the serial xbar queue) and proj units
    of block j-1 (back-loaded, after the deferred ONORM writes) are
    interleaved as fillers into attention(j)'s m-steps so the PE never
    idles waiting for ScalarE exp (and stays HAM-warm).
  - Rope outputs r live in a deep (12-buf) pool: they are only freed
    when the serial DMA-transpose queue drains, and a shallow pool
    stalls the whole Vector FIFO behind it.
"""

import sys
import threading

sys.path.insert(0, "/opt/trn_rl_repo")

import ml_dtypes
import numpy as np

import concourse.bass as bass
import concourse.mybir as mybir
from concourse import bacc
from concourse.bass_utils import run_bass_kernel_spmd
from concourse.tile import TileContext

BF16 = ml_dtypes.bfloat16
F32 = mybir.dt.float32
BF = mybir.dt.bfloat16

B, T, C = 4, 2048, 1024
NH, D = 16, 64          # global heads
HL = 8                  # local heads per core
G = 2                   # head groups (cores per batch)
FL = 3 * HL * D         # 1536 local qkv rows
CL = HL * D             # 512 local out channels
P = 128
TQ = 512                # query-block width
NTT = T // P            # 16 t-tiles
NBLK = T // TQ          # 4 query blocks
NPAIR = HL // 2         # 4 head pairs

DEBUG_DUMP = False


def build_nc():
    nc = bacc.Bacc("TRN2", target_bir_lowering=False, debug=False, num_devices=8)

    xT = nc.declare_dram_parameter("xT", [C, T], BF, isOutput=False)
    wqkvT = nc.declare_dram_parameter("wqkvT", [C, FL], BF, isOutput=False)
    wprojT = nc.declare_dram_parameter("wprojT", [CL, C], BF, isOutput=False)
    cos_t = nc.declare_dram_parameter("cos_t", [T, D // 2], F32, isOutput=False)
    msin_t = nc.declare_dram_parameter("msin_t", [T, D // 2], F32, isOutput=False)
    psin_t = nc.declare_dram_parameter("psin_t", [T, D // 2], F32, isOutput=False)
    y = nc.declare_dram_parameter("y", [T, C], F32, isOutput=True)
    if DEBUG_DUMP:
        qt_d = nc.declare_dram_parameter("qt_d", [P, NPAIR * T], BF, isOutput=True)
        kt_d = nc.declare_dram_parameter("kt_d", [P, NPAIR * T], BF, isOutput=True)
        v1_d = nc.declare_dram_parameter(
            "v1_d", [P, NTT * HL * (D + 1)], BF, isOutput=True)
        on_d = nc.declare_dram_parameter("on_d", [P, NPAIR * T], BF, isOutput=True)

    Exp = mybir.ActivationFunctionType.Exp

    with TileContext(nc) as tc:
        with (
            tc.tile_pool(name="const", bufs=1) as const,
            tc.tile_pool(name="work", bufs=6) as work,
            tc.tile_pool(name="rp", bufs=12) as rpool,
            tc.tile_pool(name="pt", bufs=4) as ptp,
            tc.tile_pool(name="small", bufs=4) as small,
            tc.tile_pool(name="ysb", bufs=3) as ysbp,
            tc.tile_pool(name="psmm", bufs=2, space="PSUM") as psmm,
            tc.tile_pool(name="psst", bufs=2, space="PSUM") as psst,
            tc.tile_pool(name="psout", bufs=2, space="PSUM") as psout,
        ):
            # ---- persistent SBUF tensors ----
            XT = const.tile([P, C // P, T], BF, tag="XT")
            WQKV = const.tile([P, C // P, FL], BF, tag="WQKV")
            WPROJ = const.tile([P, CL // P, C], BF, tag="WPROJ")
            COS = const.tile([P, NTT, D // 2], F32, tag="COS")
            MSIN = const.tile([P, NTT, D // 2], F32, tag="MSIN")
            PSIN = const.tile([P, NTT, D // 2], F32, tag="PSIN")
            V1 = const.tile([P, NTT, HL, D + 1], BF, tag="V1")
            QTp = [const.tile([P, T], BF, tag=f"QT{p}", name=f"QTp{p}")
                   for p in range(NPAIR)]
            KTp = [const.tile([P, T], BF, tag=f"KT{p}", name=f"KTp{p}")
                   for p in range(NPAIR)]
            ONORM = const.tile([P, NPAIR, T], BF, tag="ONORM")

            # input loads all on the scalar HWDGE queue so the sync queue
            # is dedicated to the DMA transposes (+ y stores)
            xTr = xT.rearrange("(ko p) t -> p ko t", p=P)
            wqr = wqkvT.rearrange("(ko p) f -> p ko f", p=P)
            # XT loaded t-block-major (block 0 first: the prologue only
            # needs block-0 columns), WQKV q/k/v-part-major on the other
            # queue — the first qkv unit is fully fed within ~5us.
            for tb in range(NBLK):
                for ko in range(C // P):
                    nc.sync.dma_start(
                        XT[:, ko, tb * TQ:(tb + 1) * TQ],
                        xTr[:, ko, tb * TQ:(tb + 1) * TQ])
            for part in range(3):
                for ko in range(C // P):
                    nc.scalar.dma_start(
                        WQKV[:, ko, part * 512:(part + 1) * 512],
                        wqr[:, ko, part * 512:(part + 1) * 512])
            nc.scalar.dma_start(COS[:], cos_t.rearrange("(n p) d -> p n d", p=P))
            nc.scalar.dma_start(MSIN[:], msin_t.rearrange("(n p) d -> p n d", p=P))
            nc.scalar.dma_start(PSIN[:], psin_t.rearrange("(n p) d -> p n d", p=P))
            nc.scalar.dma_start(
                WPROJ[:], wprojT.rearrange("(ko p) o -> p ko o", p=P))
            nc.gpsimd.memset(V1[:], 1.0)

            yr = y.rearrange("(n p) o -> p n o", p=P)

            # ---------------- pipeline unit emitters ----------------
            tr_defer = None  # when a list: (r, dst, i) collected, not emitted

            def emit_qkv_unit(i, j3):
                """qkv matmul group for t-tile i, j3 in {0:q 1:k 2:v} +
                RoPE / evacuation + DMA transposes into QT/KT."""
                ps = psmm.tile([P, 512], F32, tag="mm")
                for ko in range(C // P):
                    nc.tensor.matmul(
                        ps[:],
                        lhsT=XT[:, ko, i * P:(i + 1) * P],
                        rhs=WQKV[:, ko, j3 * 512:(j3 + 1) * 512],
                        start=(ko == 0),
                        stop=(ko == C // P - 1),
                    )
                if j3 < 2:
                    ps4 = ps.rearrange("p (h e d) -> p h e d", h=HL, e=2)
                    cosb = COS[:, i, :].unsqueeze(1).unsqueeze(1).to_broadcast(
                        [P, HL, 2, D // 2])
                    msb = MSIN[:, i, :].unsqueeze(1).to_broadcast([P, HL, D // 2])
                    psb = PSIN[:, i, :].unsqueeze(1).to_broadcast([P, HL, D // 2])
                    a = work.tile([P, 512], BF, tag="ropeA")
                    a4 = a.rearrange("p (h e d) -> p h e d", h=HL, e=2)
                    b = work.tile([P, 512], BF, tag="ropeB")
                    b4 = b.rearrange("p (h e d) -> p h e d", h=HL, e=2)
                    nc.vector.tensor_mul(a4[:], ps4[:], cosb)
                    nc.vector.tensor_mul(b4[:, :, 0, :], ps4[:, :, 1, :], msb)
                    nc.vector.tensor_mul(b4[:, :, 1, :], ps4[:, :, 0, :], psb)
                    r = rpool.tile([P, 512], BF, tag="ropeR")
                    if tr_defer is not None:  # prologue: gpsimd is idle
                        nc.gpsimd.tensor_add(r[:], a[:], b[:])
                    else:
                        nc.vector.tensor_add(r[:], a[:], b[:])
                    dst = QTp if j3 == 0 else KTp
                    if tr_defer is not None:
                        tr_defer.append((r, dst, i))
                    else:
                        for p4 in range(NPAIR):
                            nc.sync.dma_start_transpose(
                                dst[p4][:, i * P:(i + 1) * P],
                                r[:, p4 * P:(p4 + 1) * P],
                            )
                else:
                    ps3 = ps.rearrange("p (h d) -> p h d", h=HL)
                    nc.vector.tensor_copy(V1[:, i, :, 0:D], ps3[:])

            def emit_proj_unit(i, n2):
                """projection for t-tile i, output half n2."""
                ps = psmm.tile([P, 512], F32, tag="mm")
                for kc in range(NPAIR):
                    nc.tensor.matmul(
                        ps[:],
                        lhsT=ONORM[:, kc, i * P:(i + 1) * P],
                        rhs=WPROJ[:, kc, n2 * 512:(n2 + 1) * 512],
                        start=(kc == 0),
                        stop=(kc == NPAIR - 1),
                    )
                ysb = ysbp.tile([P, 512], F32, tag="ysb")
                nc.vector.tensor_copy(ysb[:], ps[:])
                nc.sync.dma_start(yr[:, i, n2 * 512:(n2 + 1) * 512], ysb[:])

            # ---- prologue: qkv for block 0, q/k first so their
            # transposes hit the serial xbar queue as early as possible
            for i in range(TQ // P):
                for j3 in range(2):
                    emit_qkv_unit(i, j3)
            for i in range(TQ // P):
                emit_qkv_unit(i, 2)

            # deferred normalization ops: popped two per m-step so each
            # engine reaches them with deps satisfied (no FIFO blocking)
            norm_pending = []

            def make_norm_ops(outU, p4, w, qsl):
                state = {}

                def s1():
                    lrow = small.tile([1, TQ], F32, tag="lrow")
                    nc.vector.tensor_copy(lrow[:], outU[D:D + 1, :])
                    state["lrow"] = lrow

                def s2():
                    r_row = small.tile([1, TQ], F32, tag="rrow")
                    nc.vector.reciprocal_approx_fast(
                        out=r_row[:], in_=state.pop("lrow")[:])
                    state["rrow"] = r_row

                def s3():
                    r64 = small.tile([D, TQ], F32, tag="rsb")
                    nc.gpsimd.partition_broadcast(r64[:], state.pop("rrow")[:])
                    state["r64"] = r64

                def s4():
                    nc.vector.tensor_mul(
                        ONORM[w * D:(w + 1) * D, p4, qsl],
                        outU[0:D, :], state.pop("r64")[:])

                return [s1, s2, s3, s4]

            # ---- attention blocks with interleaved fillers ----
            for j in range(NBLK):
                fillers = []
                if j + 1 < NBLK:
                    for i in range(TQ // P * (j + 1), TQ // P * (j + 2)):
                        for j3 in range(3):
                            fillers.append((emit_qkv_unit, i, j3))
                if j > 0:
                    for i in range(TQ // P * (j - 1), TQ // P * j):
                        for n2 in range(C // 512):
                            fillers.append((emit_proj_unit, i, n2))

                ntk = (TQ // P) * (j + 1)
                qsl = slice(j * TQ, (j + 1) * TQ)
                n_steps = NPAIR * ntk
                # qkv fillers front-loaded (their DMA transposes must land
                # before the next block's scores); proj fillers in the
                # second half (after the deferred ONORM writes settle).
                qkv_f = [f for f in fillers if f[0] is emit_qkv_unit]
                proj_f = [f for f in fillers if f[0] is emit_proj_unit]
                sched = {}
                span_q = max(n_steps // 4 if j == 0 else n_steps // 2, 1)
                for f in range(len(qkv_f)):
                    s = f * span_q // max(len(qkv_f), 1)
                    sched.setdefault(s, []).append(qkv_f[f])
                for f in range(len(proj_f)):
                    s = n_steps // 2 + f * (n_steps // 2) // max(len(proj_f), 1)
                    sched.setdefault(s, []).append(proj_f[f])

                step = 0
                for p4 in range(NPAIR):
                    outA = psout.tile([D + 1, TQ], F32, tag="out")
                    outB = psout.tile([D + 1, TQ], F32, tag="out")
                    sts = {}
                    pabs = {}

                    def emit_scores(m):
                        # diagonal-straddling tiles only need queries
                        # tq >= 128*ml: trim the free dim (causal skip)
                        ml = m - (TQ // P) * j
                        off = P * ml if ml >= 1 else 0
                        ksl = slice(m * P, (m + 1) * P)
                        qso = slice(j * TQ + off, (j + 1) * TQ)
                        st = psst.tile([P, 2 * TQ], F32, tag="st")
                        nc.tensor.matmul(
                            st[:, off:TQ], lhsT=KTp[p4][0:D, ksl],
                            rhs=QTp[p4][0:D, qso], start=True, stop=True)
                        nc.tensor.matmul(
                            st[:, TQ + off:2 * TQ], lhsT=KTp[p4][D:P, ksl],
                            rhs=QTp[p4][D:P, qso], start=True, stop=True,
                            tile_position=(D, 0))
                        sts[m] = st

                    def emit_exp(m):
                        st = sts.pop(m)
                        ml = m - (TQ // P) * j
                        off = P * ml if ml >= 1 else 0
                        pab = ptp.tile([P, 2 * TQ], BF, tag="pab")
                        sv = st.rearrange("p (e q) -> p e q", e=2)
                        pv = pab.rearrange("p (e q) -> p e q", e=2)
                        nc.scalar.activation(
                            pv[:, :, off:TQ], sv[:, :, off:TQ], Exp, scale=0.125)
                        if ml >= 0:  # diagonal-straddling: zero tk > tq
                            nc.gpsimd.affine_select(
                                out=pv[:, :, off:TQ], in_=pv[:, :, off:TQ],
                                compare_op=mybir.AluOpType.is_ge,
                                fill=0.0, base=off - P * ml,
                                pattern=[[0, 2], [1, TQ - off]],
                                channel_multiplier=-1)
                        pabs[m] = (pab, off)

                    def emit_attnv(m):
                        pab, off = pabs.pop(m)
                        nc.tensor.matmul(
                            outA[:, off:TQ], lhsT=V1[:, m, 2 * p4, :],
                            rhs=pab[:, off:TQ],
                            start=(m == 0), stop=(m == ntk - 1))
                        nc.tensor.matmul(
                            outB[:, off:TQ], lhsT=V1[:, m, 2 * p4 + 1, :],
                            rhs=pab[:, TQ + off:2 * TQ],
                            start=(m == 0), stop=(m == ntk - 1))

                    emit_scores(0)
                    for m in range(ntk):
                        emit_exp(m)
                        if m + 1 < ntk:
                            emit_scores(m + 1)
                        if m >= 2:
                            for _ in range(2):
                                if norm_pending:
                                    norm_pending.pop(0)()
                        for fn_args in sched.pop(step, []):
                            if fn_args[0] is emit_proj_unit:
                                while norm_pending:  # proj reads ONORM
                                    norm_pending.pop(0)()
                            fn_args[0](*fn_args[1:])
                        if m >= 2:
                            emit_attnv(m - 2)  # 2-step lag hides the mask
                        step += 1
                    emit_attnv(ntk - 2)
                    emit_attnv(ntk - 1)

                    # stage PSUM out to SBUF on ScalarE (its queue is
                    # shallow here, so the banks free in ~1us); the rest
                    # of the normalization is deferred into the following
                    # pair's m-steps.
                    for w, outp in ((0, outA), (1, outB)):
                        outU = small.tile([D + 1, TQ], F32, tag="outU")
                        if w == 0:
                            nc.scalar.copy(out=outU[:], in_=outp[:])
                        else:
                            nc.vector.tensor_copy(outU[:], outp[:])
                        norm_pending.extend(make_norm_ops(outU, p4, w, qsl))
                for s in sorted(sched):  # safety: unreached fillers
                    for fn_args in sched[s]:
                        fn_args[0](*fn_args[1:])

            while norm_pending:  # drain the last pair's normalization
                norm_pending.pop(0)()

            # ---- epilogue: proj for the last block ----
            for i in range(TQ // P * (NBLK - 1), TQ // P * NBLK):
                for n2 in range(C // 512):
                    emit_proj_unit(i, n2)

            if DEBUG_DUMP:
                qdv = qt_d.rearrange("p (a t) -> p a t", a=NPAIR)
                kdv = kt_d.rearrange("p (a t) -> p a t", a=NPAIR)
                for p in range(NPAIR):
                    nc.sync.dma_start(qdv[:, p, :], QTp[p][:])
                    nc.sync.dma_start(kdv[:, p, :], KTp[p][:])
                nc.sync.dma_start(v1_d[:], V1.rearrange("p a h d -> p (a h d)"))
                nc.sync.dma_start(on_d[:], ONORM.rearrange("p a t -> p (a t)"))

    nc.compile()
    return nc


def prep_inputs(x, w_qkv, w_proj):
    """Build the 8 per-core input maps from the full-problem inputs."""
    x = np.asarray(x, dtype=np.float32)
    w_qkv = np.asarray(w_qkv, dtype=np.float32)
    w_proj = np.asarray(w_proj, dtype=np.float32)

    inv_freq = 1.0 / (10000.0 ** (np.arange(0, D, 2, dtype=np.float32) / D))
    tt = np.arange(T, dtype=np.float32)
    freqs = np.outer(tt, inv_freq).astype(np.float32)  # [T, 32]
    cos_t = np.cos(freqs).astype(np.float32)
    sin_t = np.sin(freqs).astype(np.float32)
    msin_t = (-sin_t).astype(np.float32)

    in_maps = []
    for core in range(8):
        b, g = divmod(core, G)
        sl = slice(g * CL, (g + 1) * CL)
        w_local = np.concatenate(
            [w_qkv[sl], w_qkv[C:][sl], w_qkv[2 * C:][sl]], axis=0)  # [1536, C]
        in_maps.append({
            "xT": np.ascontiguousarray(x[b].T).astype(BF16),
            "wqkvT": np.ascontiguousarray(w_local.T).astype(BF16),
            "wprojT": np.ascontiguousarray(w_proj[:, sl].T).astype(BF16),
            "cos_t": cos_t,
            "msin_t": msin_t,
            "psin_t": sin_t,
        })
    return in_maps


_NC_LOCK = threading.Lock()
_NC = None


def get_nc():
    global _NC
    with _NC_LOCK:
        if _NC is None:
            _NC = build_nc()
    return _NC


def run(nc, in_maps, **kw):
    res = run_bass_kernel_spmd(nc, in_maps, list(range(8)), **kw)
    parts = [res.results[c]["y"] for c in range(8)]
    out = np.stack([parts[2 * b] + parts[2 * b + 1] for b in range(B)])
    return out.astype(np.float32), res


def kernel(x, w_qkv, w_proj):
    out, _ = run(get_nc(), prep_inputs(x, w_qkv, w_proj))
    return out
